# revision 1
# baseline (speedup 1.0000x reference)
"""Causal self-attention (B=4, T=2048, C=768, H=12) on 8 trn2 NeuronCores.

Sharding: core c -> batch b = c//2, head-half hh = c%2 (6 heads per core).
Each core computes, for its (b, 6 heads): qkv projection, causal attention,
and a partial output projection (its heads' rows of W_proj). The host sums
the two partial outputs per batch and adds b_proj.

All matmuls run in float32r (tf32-like, ~13-bit mantissa, full PE rate at
N>=256). Layouts are chosen so the PE contracts over partitions everywhere
and softmax needs no transposes:
  - q^T, k^T [d, T]: weight-stationary qkv matmul
  - S^T [tk, tq] blocks: lhsT = k^T tile, rhs = q^T chunk; two heads of a
    pair run concurrently via row groups (base partitions 0/64, K=64)
  - exp on ACT straight out of PSUM ([128, 1024] pair tiles, causal-skipped)
  - PV: stationary [v_A | ones | v_B] per pair; the ones block makes each
    head's softmax denominator come out replicated on the 64 partitions
    opposite its O^T rows, so normalize = reciprocal + partition-swap DMA +
    elementwise multiply, no cross-partition vector ops.
"""

import numpy as np

B, T, C = 4, 2048, 768
H = 12
D = C // H          # 64
HPC = 6             # heads per core
NP = 3              # head pairs per core
N_CORES = 8
TK = T // 128       # 16 tk tiles
NCH = T // 512      # 4 tq chunks
CT = C // 128       # 6 contraction tiles

_cache = {}


def _build(has_bias):
    import concourse.tile as tile
    from concourse import bacc, mybir

    dt = mybir.dt
    r32 = dt.float32r
    f32 = dt.float32
    bf16 = dt.bfloat16
    Exp = mybir.ActivationFunctionType.Exp

    nc = bacc.Bacc("TRN2", target_bir_lowering=False, debug=False,
                   num_devices=N_CORES)

    xT_ap = nc.dram_tensor("xT", [C, T], r32, kind="ExternalInput").ap()
    wa_ap = nc.dram_tensor("wa", [C, 1152], r32, kind="ExternalInput").ap()
    wp_ap = nc.dram_tensor("wp", [HPC * D, C], r32, kind="ExternalInput").ap()
    tri_ap = nc.dram_tensor("tri", [128, 128], f32, kind="ExternalInput").ap()
    if has_bias:
        ox_ap = nc.dram_tensor("ox", [1, T], r32, kind="ExternalInput").ap()
        wb_ap = nc.dram_tensor("wb", [1, 1152], r32, kind="ExternalInput").ap()
    out_ap = nc.dram_tensor("out", [T, C], f32, kind="ExternalOutput").ap()
    import os as _os
    dbg = bool(_os.environ.get("KV_DEBUG"))
    if dbg:
        dq_ap = nc.dram_tensor("dq", [128, T], r32, kind="ExternalOutput").ap()
        dk_ap = nc.dram_tensor("dk", [128, T], r32, kind="ExternalOutput").ap()
        pass
        dy_ap = nc.dram_tensor("dy", [128, T], r32, kind="ExternalOutput").ap()
        dpa_ap = nc.dram_tensor("dpa", [128, 512], f32, kind="ExternalOutput").ap()
        dpb_ap = nc.dram_tensor("dpb", [128, 512], f32, kind="ExternalOutput").ap()
        ds1_ap = nc.dram_tensor("ds1", [128, 512], f32, kind="ExternalOutput").ap()
        ds2_ap = nc.dram_tensor("ds2", [128, 512], f32, kind="ExternalOutput").ap()
        dP_ap = nc.dram_tensor("dP", [128, 1024], r32, kind="ExternalOutput").ap()

    with tile.TileContext(nc) as tc:
        with tc.tile_pool(name="pers", bufs=1) as pers, \
             tc.tile_pool(name="pP", bufs=3) as pP, \
             tc.tile_pool(name="pst", bufs=2) as pst, \
             tc.tile_pool(name="pout", bufs=2) as pout, \
             tc.tile_pool(name="psA", bufs=2, space="PSUM") as psA, \
             tc.tile_pool(name="psB", bufs=4, space="PSUM") as psB:

            # ---- persistent SBUF tensors + input DMA ----
            xT = [pers.tile([128, T], r32, tag=f"x{i}", name=f"x{i}") for i in range(CT)]
            wa = [pers.tile([128, 1152], r32, tag=f"w{i}", name=f"w{i}") for i in range(CT)]
            for i in range(CT):
                nc.sync.dma_start(xT[i][:], xT_ap[i * 128:(i + 1) * 128, :])
                nc.sync.dma_start(wa[i][:], wa_ap[i * 128:(i + 1) * 128, :])
            wp = [pers.tile([128, C], r32, tag=f"p{i}", name=f"wp{i}") for i in range(NP)]
            for i in range(NP):
                nc.sync.dma_start(wp[i][:], wp_ap[i * 128:(i + 1) * 128, :])
            tri = pers.tile([128, 128], f32, tag="tri")
            nc.sync.dma_start(tri[:], tri_ap)
            if has_bias:
                ox = pers.tile([1, T], r32, tag="ox")
                nc.sync.dma_start(ox[:], ox_ap)
                wb = pers.tile([1, 1152], r32, tag="wb")
                nc.sync.dma_start(wb[:], wb_ap)
            ones_f = pers.tile([128, 64], f32, tag="onesf")
            nc.vector.memset(ones_f[:], 1.0)
            ones_b = pers.tile([128, 64], bf16, tag="onesb")
            nc.vector.tensor_copy(ones_b[:], ones_f[:])
            tri_b = pers.tile([128, 128], bf16, tag="trib")
            nc.vector.tensor_copy(tri_b[:], tri[:])

            qk = [pers.tile([128, T], r32, tag=f"qk{i}", name=f"qk{i}") for i in range(2 * NP)]
            # v: one contiguous [128, 64] bf16 tile per (t-tile, head) — bf16
            # stationaries must be dense tiles (sliced wide tiles hit a slow
            # non-FWL weight-load path).
            vv = [[pers.tile([128, 64], bf16, tag=f"v{t}_{h}", name=f"v{t}_{h}")
                   for h in range(HPC)] for t in range(TK)]
            # Y^T for pair p: fresh tile for p=0; pairs 1 and 2 reuse the
            # q^T tiles of pairs 0 and 1, which are dead by the time attn
            # p starts writing (Tile tracks the WAR dependency).
            y0 = pers.tile([128, T], r32, tag="y0", name="y0")
            yt = [y0, qk[0], qk[2]]

            # ---- phase 1: qkv projections ----
            # q^T / k^T: stationary = wa column block, moving = xT chunk.
            # out tile order: pair0 q, pair0 k, pair1 q, ... so attention on
            # pair p can start as early as possible.
            with nc.named_scope("qkv_qk"):
                for p in range(NP):
                    for qsel in range(2):          # 0 = q, 1 = k
                        dst = qk[2 * p + qsel]
                        wcol = qsel * 384 + p * 128
                        for cp in range(2):        # chunk pairs (1024 cols)
                            ps = psA.tile([128, 1024], f32, tag="A")
                            for half in range(2):
                                t0 = cp * 1024 + half * 512
                                for c in range(CT):
                                    nc.tensor.matmul(
                                        ps[:, half * 512:half * 512 + 512],
                                        lhsT=wa[c][:, wcol:wcol + 128],
                                        rhs=xT[c][:, t0:t0 + 512],
                                        start=(c == 0),
                                        stop=(c == CT - 1 and not has_bias))
                                if has_bias:
                                    nc.tensor.matmul(
                                        ps[:, half * 512:half * 512 + 512],
                                        lhsT=wb[0:1, wcol:wcol + 128],
                                        rhs=ox[0:1, t0:t0 + 512],
                                        start=False, stop=True)
                            nc.vector.tensor_copy(
                                dst[:, cp * 1024:(cp + 1) * 1024], ps[:])

            # v: stationary = xT[c] t-tile, moving = wa v columns.
            with nc.named_scope("qkv_v"):
                for t in range(TK):
                    ps = psB.tile([128, 512], f32, tag="B")
                    for c in range(CT):
                        nc.tensor.matmul(
                            ps[:, 0:384],
                            lhsT=xT[c][:, t * 128:(t + 1) * 128],
                            rhs=wa[c][:, 768:1152],
                            start=(c == 0),
                            stop=(c == CT - 1 and not has_bias))
                    if has_bias:
                        nc.tensor.matmul(
                            ps[:, 0:384],
                            lhsT=ox[0:1, t * 128:(t + 1) * 128],
                            rhs=wb[0:1, 768:1152],
                            start=False, stop=True)
                    for h in range(HPC):
                        nc.vector.tensor_copy(
                            vv[t][h][:], ps[:, h * 64:(h + 1) * 64])

            if dbg:
                nc.sync.dma_start(dq_ap, qk[0][:])
                nc.sync.dma_start(dk_ap, qk[1][:])

            # ---- phase 2: attention per head pair ----
            for p in range(NP):
                qA = qk[2 * p]
                kA = qk[2 * p + 1]
                with nc.named_scope(f"attn{p}"):
                    for j in range(NCH):
                        nblk = 4 * j + 4
                        pvA = psB.tile([128, 512], f32, tag="B")
                        pvB = psB.tile([128, 512], f32, tag="B")
                        Ps = [None] * nblk
                        ms = [None] * nblk

                        def emit_S(i):
                            m = i - 4 * j
                            lo = 128 * m if m >= 0 else 0
                            w = 512 - lo
                            sp = psA.tile([128, 1024], f32, tag="A")
                            for ab in range(2):
                                nc.tensor.matmul(
                                    sp[:, ab * 512 + lo:(ab + 1) * 512],
                                    lhsT=kA[ab * 64:(ab + 1) * 64,
                                            i * 128:(i + 1) * 128],
                                    rhs=qA[ab * 64:(ab + 1) * 64,
                                           j * 512 + lo:(j + 1) * 512],
                                    start=True, stop=True)
                            P = pP.tile([128, 1024], bf16, tag="P")
                            if lo:
                                nc.scalar.activation(
                                    P[:, lo:512], sp[:, lo:512], Exp)
                                nc.scalar.activation(
                                    P[:, 512 + lo:1024], sp[:, 512 + lo:1024],
                                    Exp)
                            else:
                                nc.scalar.activation(P[:], sp[:], Exp)
                            Ps[i], ms[i] = P, max(m, 0)

                        def emit_PV(i):
                            m = ms[i]
                            lo = 128 * m
                            P = Ps[i]
                            if m > 0 or i == 4 * j:
                                # diagonal sub-block masking (multiply by tri)
                                for ab in range(2):
                                    sl = P[:, ab * 512 + lo:ab * 512 + lo + 128]
                                    nc.vector.tensor_mul(sl, sl, tri_b[:])
                            first, last = (i == 0), (i == nblk - 1)
                            # head A: O on partitions 0-63, denom (ones) on
                            # 64-127; head B mirrored. O and denom matmuls
                            # land in different column groups and overlap.
                            nc.tensor.matmul(
                                pvA[0:64, lo:512], lhsT=vv[i][2 * p][:],
                                rhs=P[:, lo:512], start=first, stop=last)
                            nc.tensor.matmul(
                                pvA[64:128, lo:512], lhsT=ones_b[:],
                                rhs=P[:, lo:512], start=first, stop=last)
                            nc.tensor.matmul(
                                pvB[0:64, lo:512], lhsT=ones_b[:],
                                rhs=P[:, 512 + lo:1024], start=first, stop=last)
                            nc.tensor.matmul(
                                pvB[64:128, lo:512], lhsT=vv[i][2 * p + 1][:],
                                rhs=P[:, 512 + lo:1024], start=first, stop=last)

                        # software-pipeline: S(i+1) is emitted before PV(i)
                        emit_S(0)
                        for i in range(1, nblk):
                            emit_S(i)
                            emit_PV(i - 1)
                        emit_PV(nblk - 1)

                        # normalize. reciprocal_approx_fast only works at
                        # base partition 0, so: head B's denom (rows 0-63)
                        # is recip'd in place; head A's denom (rows 64-127)
                        # is staged, swapped down via DMA, then recip'd.
                        s1 = pst.tile([128, 512], f32, tag="st")
                        nc.vector.tensor_copy(s1[64:128, :], pvA[64:128, :])
                        nc.vector.reciprocal_approx_fast(
                            s1[0:64, :], pvB[0:64, :])
                        s2 = pst.tile([128, 512], f32, tag="rc")
                        nc.sync.dma_start(s2[0:64, :], s1[64:128, :])
                        nc.sync.dma_start(s2[64:128, :], s1[0:64, :])
                        s3 = pst.tile([64, 512], f32, tag="s3")
                        nc.vector.reciprocal_approx_fast(
                            s3[:], s2[0:64, :])
                        if dbg and p == 0 and j == 0:
                            dpa = pout.tile([128, 512], f32, tag="o", name="dpa")
                            nc.vector.tensor_copy(dpa[:], pvA[:])
                            nc.sync.dma_start(dpa_ap, dpa[:])
                            dpb = pout.tile([128, 512], f32, tag="o", name="dpb")
                            nc.vector.tensor_copy(dpb[:], pvB[:])
                            nc.sync.dma_start(dpb_ap, dpb[:])
                            nc.sync.dma_start(ds1_ap, s1[:])
                            nc.sync.dma_start(ds2_ap, s2[:])
                            nc.sync.dma_start(dP_ap, Ps[0][:])
                        nc.vector.tensor_mul(
                            yt[p][0:64, j * 512:(j + 1) * 512],
                            pvA[0:64, :], s3[:])
                        nc.vector.tensor_mul(
                            yt[p][64:128, j * 512:(j + 1) * 512],
                            pvB[64:128, :], s2[64:128, :])

            if dbg:
                nc.sync.dma_start(dy_ap, yt[0][:])

            # ---- phase 3: output projection (partial; host adds b_proj) ----
            with nc.named_scope("proj"):
                for t in range(TK):
                    ps = psA.tile([128, 1024], f32, tag="A")
                    for n0, n1 in ((0, 512), (512, 768)):
                        for kk in range(NP):
                            nc.tensor.matmul(
                                ps[:, n0:n1],
                                lhsT=yt[kk][:, t * 128:(t + 1) * 128],
                                rhs=wp[kk][:, n0:n1],
                                start=(kk == 0), stop=(kk == NP - 1))
                    ob = pout.tile([128, C], f32, tag="o")
                    nc.vector.tensor_copy(ob[:], ps[:, 0:C])
                    nc.sync.dma_start(
                        out_ap[t * 128:(t + 1) * 128, :], ob[:])

    nc.compile()
    return nc


def _prep_inputs(x, W_qkv, b_qkv, W_proj):
    """Per-core input maps (numpy, float32 bits; fp32r tensors reuse them)."""
    sc = 1.0 / np.sqrt(D)
    tri = np.triu(np.ones((128, 128), dtype=np.float32))
    in_maps = []
    for c in range(N_CORES):
        b, hh = c // 2, c % 2
        h0 = hh * 384                      # column offset of this half's heads
        wq = W_qkv[:, h0:h0 + 384] * sc
        wk = W_qkv[:, 768 + h0:768 + h0 + 384]
        wv = W_qkv[:, 1536 + h0:1536 + h0 + 384]
        wa = np.ascontiguousarray(
            np.concatenate([wq, wk, wv], axis=1), dtype=np.float32)
        m = {
            "xT": np.ascontiguousarray(x[b].T, dtype=np.float32),
            "wa": wa,
            "wp": np.ascontiguousarray(W_proj[h0:h0 + 384, :], np.float32),
            "tri": tri,
        }
        if np.any(b_qkv):
            bq = b_qkv[h0:h0 + 384] * sc
            bk = b_qkv[768 + h0:768 + h0 + 384]
            bv = b_qkv[1536 + h0:1536 + h0 + 384]
            m["ox"] = np.ones((1, T), dtype=np.float32)
            m["wb"] = np.concatenate([bq, bk, bv]).reshape(1, 1152).astype(
                np.float32)
        in_maps.append(m)
    return in_maps


def _run(inputs, trace=False, tmpdir=None):
    from concourse.bass_utils import run_bass_kernel_spmd

    x = np.asarray(inputs["x"], dtype=np.float32)
    W_qkv = np.asarray(inputs["W_qkv"], dtype=np.float32)
    b_qkv = np.asarray(inputs["b_qkv"], dtype=np.float32)
    W_proj = np.asarray(inputs["W_proj"], dtype=np.float32)
    b_proj = np.asarray(inputs["b_proj"], dtype=np.float32)

    has_bias = bool(np.any(b_qkv))
    key = ("k", has_bias)
    if key not in _cache:
        _cache[key] = _build(has_bias)
    nc = _cache[key]

    in_maps = _prep_inputs(x, W_qkv, b_qkv, W_proj)
    res = run_bass_kernel_spmd(nc, in_maps, list(range(N_CORES)),
                               trace=trace, tmpdir=tmpdir)
    out = np.empty((B, T, C), dtype=np.float32)
    for b in range(B):
        out[b] = res.results[2 * b]["out"] + res.results[2 * b + 1]["out"]
    out += b_proj
    return out, res


def kernel(**inputs):
    out, _ = _run(inputs)
    return out

